# revision 7
# baseline (speedup 1.0000x reference)
"""Trainium2 Bass kernel for nn_ActorNetSpiking (2-layer spiking LIF MLP, T=16).

Strategy (8 NeuronCores, data-parallel over batch):
  - Each core gets BL=512 batch rows; weights replicated.
  - Feature-major layout on chip: states are [feature -> chunks of 128
    partitions, BL free].  vd stores 0.75*v*(1-s) (f16) so the recurrence is
    v' = vd + u'.  Spikes are handled as q = (v <= 0.5)*0.75 in f16, with
    s = 1 - q/0.75; layer-2 matmul uses q against W2/0.75 (f16) and folds
    colsum(W2) + b2 into the bias, with a subtract.
  - mm1 (x @ W1.T) in fp32 (precision-mandatory: f16 inputs flip spikes).
    mm2 (s1 @ W2.T) in f16 (binary spikes exact in f16; tested safe).
  - Prologue PE-transposes x, W1, W2 into blocked DRAM scratch layouts.
"""
import numpy as np

import concourse.bacc as bacc
import concourse.bass as bass
import concourse.tile as tile
from concourse import masks, mybir
from concourse.bass_utils import run_bass_kernel_spmd

F32 = mybir.dt.float32
F16 = mybir.dt.float16
AL = mybir.AluOpType

NCORES = 8
B, S, H, A, T = 4096, 1024, 4096, 512, 16
BL = B // NCORES          # 512 batch rows per core
KC = S // 128             # 8 contraction chunks for mm1
MC = H // 128             # 32 hidden chunks
AC = A // 128             # 4 action chunks
BC = BL // 128            # 4 batch blocks of 128

MM1_DT = F32              # flip to mybir.dt.float32r to A/B the fast path

_CACHED_NC = None


def _build():
    nc = bacc.Bacc()
    x_p = nc.declare_dram_parameter("x", [BL, S, T], F32, isOutput=False)
    w1_p = nc.declare_dram_parameter("w1", [H, S], F32, isOutput=False)
    b1_p = nc.declare_dram_parameter("b1", [H], F32, isOutput=False)
    w2_p = nc.declare_dram_parameter("w2", [A, H], F32, isOutput=False)
    b2_p = nc.declare_dram_parameter("b2", [A], F32, isOutput=False)
    out1_p = nc.declare_dram_parameter("out1", [H, BL], F32, isOutput=True)
    out2_p = nc.declare_dram_parameter("out2", [A, BL], F32, isOutput=True)

    with tile.TileContext(nc) as tc:
        with tc.tile_pool(name="persist", bufs=1) as persist, \
             tc.tile_pool(name="dram_pool", bufs=1, space="DRAM") as dram_pool, \
             tc.tile_pool(name="ps1_pool", bufs=2, space="PSUM") as ps1_pool, \
             tc.tile_pool(name="ps2_pool", bufs=1, space="PSUM") as ps2_pool:

            # DRAM scratch, blocked layouts (as tiles so Tile tracks RAW deps)
            xt_d = dram_pool.tile([T, KC, 128, BL], F32, name="xt_d")
            w1t_d = dram_pool.tile([MC, KC, 128, 128], F32, name="w1t_d")
            w2t_d = dram_pool.tile([MC, 128, A], F16, name="w2t_d")

            ident = persist.tile([128, 128], F32, name="ident")
            masks.make_identity(nc, ident)

            u1 = [persist.tile([128, BL], F32, name=f"u1_{m}") for m in range(MC)]
            vd1 = [persist.tile([128, BL], F16, name=f"vd1_{m}") for m in range(MC)]
            sq1 = [persist.tile([128, BL], F16, name=f"sq1_{m}") for m in range(MC)]
            u2 = [persist.tile([128, BL], F32, name=f"u2_{a}") for a in range(AC)]
            vd2 = [persist.tile([128, BL], F16, name=f"vd2_{a}") for a in range(AC)]
            sq2 = [persist.tile([128, BL], F16, name=f"sq2_{a}") for a in range(AC)]
            b1t = persist.tile([128, MC], F32, name="b1t")
            b2c = persist.tile([128, AC], F32, name="b2c")

            for t_ in u1 + vd1 + u2 + vd2 + sq1 + sq2:
                nc.vector.memset(t_, 0.0)

            # ---------------- prologue: transposes into DRAM scratch --------
            with tc.tile_pool(name="pro_b", bufs=1) as pro_b, \
                 tc.tile_pool(name="pro_b_ps", bufs=1, space="PSUM") as pro_b_ps:
                # b1 -> b1t [128, MC]
                b1_sb = pro_b.tile([MC, 128], F32, name="b1_sb")
                nc.sync.dma_start(out=b1_sb, in_=b1_p.rearrange("(c p) -> c p", p=128))
                ps_b1 = pro_b_ps.tile([128, MC], F32, name="ps_b1")
                nc.tensor.transpose(ps_b1, b1_sb, ident[:MC, :MC])
                nc.vector.tensor_copy(b1t, ps_b1)
                # b2 -> b2c [128, AC]; then += rowsum(W2) below
                b2_sb = pro_b.tile([AC, 128], F32, name="b2_sb")
                nc.sync.dma_start(out=b2_sb, in_=b2_p.rearrange("(c p) -> c p", p=128))
                ps_b2 = pro_b_ps.tile([128, AC], F32, name="ps_b2")
                nc.tensor.transpose(ps_b2, b2_sb, ident[:AC, :AC])
                nc.vector.tensor_copy(b2c, ps_b2)

            # W2: per a-chunk: load [128, H]; c2 = rowsum; transpose blocks
            with tc.tile_pool(name="pro_w2", bufs=2) as pro_w2, \
                 tc.tile_pool(name="pro_w2_ps", bufs=2, space="PSUM") as pro_w2_ps:
                for ac in range(AC):
                    w2_chunk = pro_w2.tile([128, H], F32, name="w2_chunk",
                                           tag="w2_chunk")
                    nc.sync.dma_start(out=w2_chunk,
                                      in_=w2_p[ac * 128:(ac + 1) * 128, :])
                    c2 = pro_w2.tile([128, 1], F32, name="c2", tag="c2")
                    nc.vector.tensor_reduce(out=c2, in_=w2_chunk, op=AL.add,
                                            axis=mybir.AxisListType.X)
                    nc.vector.tensor_tensor(out=b2c[:, ac:ac + 1],
                                            in0=b2c[:, ac:ac + 1], in1=c2, op=AL.add)
                    for hc in range(MC):
                        ps_w2 = pro_w2_ps.tile([128, 128], F32, name="ps_w2",
                                               tag="ps_w2")
                        nc.tensor.transpose(
                            ps_w2, w2_chunk[:, hc * 128:(hc + 1) * 128], ident)
                        st_w2 = pro_w2.tile([128, 128], F16, name="st_w2",
                                            tag="st_w2")
                        # scale by 1/0.75 so (W2/0.75) @ (0.75*q) = W2 @ q
                        nc.scalar.activation(st_w2, ps_w2,
                                             mybir.ActivationFunctionType.Copy,
                                             bias=0.0, scale=4.0 / 3.0)
                        nc.sync.dma_start(
                            out=w2t_d[hc, :, ac * 128:(ac + 1) * 128], in_=st_w2)

            # W1: per h-chunk: load [128, S]; transpose 8 s-blocks
            with tc.tile_pool(name="pro_w1", bufs=2) as pro_w1, \
                 tc.tile_pool(name="pro_w1_ps", bufs=2, space="PSUM") as pro_w1_ps:
                for hc in range(MC):
                    w1_chunk = pro_w1.tile([128, S], F32, name="w1_chunk",
                                           tag="w1_chunk")
                    nc.sync.dma_start(out=w1_chunk,
                                      in_=w1_p[hc * 128:(hc + 1) * 128, :])
                    for g in range(2):  # two groups of 4 s-blocks
                        ps_w1 = pro_w1_ps.tile([128, 512], F32, name="ps_w1",
                                               tag="ps_w1")
                        for j in range(4):
                            sc = g * 4 + j
                            nc.tensor.transpose(
                                ps_w1[:, j * 128:(j + 1) * 128],
                                w1_chunk[:, sc * 128:(sc + 1) * 128], ident)
                        st_w1 = pro_w1.tile([128, 512], F32, name="st_w1",
                                            tag="st_w1")
                        nc.scalar.copy(st_w1, ps_w1)
                        nc.sync.dma_start(
                            out=w1t_d[hc, g * 4:(g + 1) * 4, :, :].rearrange(
                                "c s h -> s c h"),
                            in_=st_w1.rearrange("s (c h) -> s c h", c=4))

            # x: per (s-chunk, t-half): load 4 b-blocks, per t transpose 4 blocks
            TH = T // 2
            with tc.tile_pool(name="pro_x", bufs=1) as pro_x, \
                 tc.tile_pool(name="pro_x2", bufs=2) as pro_x2, \
                 tc.tile_pool(name="pro_x_ps", bufs=2, space="PSUM") as pro_x_ps:
                for sc in range(KC):
                    for th in range(2):
                        xch = [pro_x.tile([128, 128, TH], F32, name=f"xch{bc}",
                                          tag=f"xch{bc}") for bc in range(BC)]
                        for bc in range(BC):
                            nc.sync.dma_start(
                                out=xch[bc],
                                in_=x_p[bc * 128:(bc + 1) * 128,
                                        sc * 128:(sc + 1) * 128,
                                        th * TH:(th + 1) * TH])
                        for tt_ in range(TH):
                            t = th * TH + tt_
                            ps_x = pro_x_ps.tile([128, BL], F32, name="ps_x",
                                                 tag="ps_x")
                            for bc in range(BC):
                                nc.tensor.transpose(
                                    ps_x[:, bc * 128:(bc + 1) * 128],
                                    xch[bc][:, :, tt_], ident)
                            st_x = pro_x2.tile([128, BL], F32, name="st_x",
                                               tag="st_x")
                            if t % 2 == 0:
                                nc.scalar.copy(st_x, ps_x)
                            else:
                                nc.vector.tensor_copy(st_x, ps_x)
                            nc.sync.dma_start(out=xt_d[t, sc, :, :], in_=st_x)

            # ---------------- main scan over T timesteps --------------------
            with tc.tile_pool(name="xt_pool", bufs=2) as xt_pool, \
                 tc.tile_pool(name="w1_pool", bufs=3) as w1_pool, \
                 tc.tile_pool(name="w2_pool", bufs=3) as w2_pool, \
                 tc.tile_pool(name="q_pool", bufs=4) as q_pool, \
                 tc.tile_pool(name="v_pool", bufs=3) as v_pool:

                for t in range(T):
                    xt_sb = [xt_pool.tile([128, BL], F32, name=f"xt{sc}",
                                          tag=f"xt{sc}") for sc in range(KC)]
                    for sc in range(KC):
                        nc.sync.dma_start(out=xt_sb[sc], in_=xt_d[t, sc, :, :])

                    ps2 = [ps2_pool.tile([128, BL], F32, name=f"ps2_{a}",
                                         tag=f"ps2_{a}") for a in range(AC)]
                    q_tiles = {}
                    w2_tiles = {}

                    def issue_mm2(m):
                        w2_sb = w2_tiles.pop(m)
                        qm = q_tiles.pop(m)
                        for a in range(AC):
                            nc.tensor.matmul(
                                ps2[a], lhsT=w2_sb[:, a * 128:(a + 1) * 128],
                                rhs=qm, start=(m == 0), stop=(m == MC - 1))

                    for m in range(MC):
                        w1_sb = w1_pool.tile([128, S], F32, name="w1_sb",
                                             tag="w1_sb")
                        nc.sync.dma_start(
                            out=w1_sb.rearrange("s (c h) -> s c h", c=KC),
                            in_=w1t_d[m].rearrange("c s h -> s c h"))
                        w2_sb = w2_pool.tile([128, A], F16, name="w2_sb",
                                             tag="w2_sb")
                        nc.sync.dma_start(out=w2_sb, in_=w2t_d[m])
                        w2_tiles[m] = w2_sb

                        ps1 = ps1_pool.tile([128, BL], F32, name="ps1", tag="ps1")
                        for sc in range(KC):
                            if MM1_DT == F32:
                                lhsT = w1_sb[:, sc * 128:(sc + 1) * 128]
                                rhs = xt_sb[sc]
                            else:
                                lhsT = w1_sb[:, sc * 128:(sc + 1) * 128].bitcast(MM1_DT)
                                rhs = xt_sb[sc].bitcast(MM1_DT)
                            nc.tensor.matmul(ps1, lhsT=lhsT, rhs=rhs,
                                             start=(sc == 0), stop=(sc == KC - 1))

                        if m > 0:
                            issue_mm2(m - 1)

                        # LIF update for hidden chunk m
                        nc.vector.tensor_scalar(
                            out=u1[m], in0=u1[m], scalar1=0.5,
                            scalar2=b1t[:, m:m + 1], op0=AL.mult, op1=AL.add)
                        nc.vector.tensor_tensor(out=u1[m], in0=u1[m], in1=ps1,
                                                op=AL.add)
                        vtmp = v_pool.tile([128, BL], F32, name="vtmp", tag="vtmp")
                        nc.vector.tensor_tensor(out=vtmp, in0=vd1[m], in1=u1[m],
                                                op=AL.add)
                        qm = q_pool.tile([128, BL], F16, name="qm", tag="qm")
                        nc.vector.tensor_scalar(out=qm, in0=vtmp, scalar1=0.5,
                                                scalar2=0.75, op0=AL.is_le,
                                                op1=AL.mult)
                        q_tiles[m] = qm
                        nc.vector.tensor_tensor(out=vd1[m], in0=vtmp, in1=qm,
                                                op=AL.mult)
                        nc.vector.tensor_tensor(out=sq1[m], in0=sq1[m], in1=qm,
                                                op=AL.add)

                    issue_mm2(MC - 1)

                    # layer-2 LIF
                    for a in range(AC):
                        nc.vector.tensor_scalar(
                            out=u2[a], in0=u2[a], scalar1=0.5,
                            scalar2=b2c[:, a:a + 1], op0=AL.mult, op1=AL.add)
                        nc.vector.tensor_tensor(out=u2[a], in0=u2[a], in1=ps2[a],
                                                op=AL.subtract)
                        vtmp2 = v_pool.tile([128, BL], F32, name="vtmp2",
                                            tag="vtmp")
                        nc.vector.tensor_tensor(out=vtmp2, in0=vd2[a], in1=u2[a],
                                                op=AL.add)
                        q2 = q_pool.tile([128, BL], F16, name="q2", tag="q2")
                        nc.vector.tensor_scalar(out=q2, in0=vtmp2, scalar1=0.5,
                                                scalar2=0.75, op0=AL.is_le,
                                                op1=AL.mult)
                        nc.vector.tensor_tensor(out=vd2[a], in0=vtmp2, in1=q2,
                                                op=AL.mult)
                        nc.vector.tensor_tensor(out=sq2[a], in0=sq2[a], in1=q2,
                                                op=AL.add)

            # epilogue: out = 1 - sumq/12   (sumq = 0.75 * sum q; T=16)
            with tc.tile_pool(name="out_pool", bufs=2) as out_pool:
                for m in range(MC):
                    o1 = out_pool.tile([128, BL], F32, name="o1", tag="o1")
                    nc.vector.tensor_scalar(out=o1, in0=sq1[m],
                                            scalar1=-1.0 / 12.0, scalar2=1.0,
                                            op0=AL.mult, op1=AL.add)
                    nc.sync.dma_start(out=out1_p[m * 128:(m + 1) * 128, :], in_=o1)
                for a in range(AC):
                    o2 = out_pool.tile([128, BL], F32, name="o2", tag="o2")
                    nc.vector.tensor_scalar(out=o2, in0=sq2[a],
                                            scalar1=-1.0 / 12.0, scalar2=1.0,
                                            op0=AL.mult, op1=AL.add)
                    nc.sync.dma_start(out=out2_p[a * 128:(a + 1) * 128, :], in_=o2)

    nc.finalize()
    return nc


def get_nc():
    global _CACHED_NC
    if _CACHED_NC is None:
        _CACHED_NC = _build()
    return _CACHED_NC


def make_in_maps(x, W1, b1, W2, b2):
    x = np.ascontiguousarray(np.asarray(x, np.float32))
    W1 = np.ascontiguousarray(np.asarray(W1, np.float32))
    b1 = np.ascontiguousarray(np.asarray(b1, np.float32))
    W2 = np.ascontiguousarray(np.asarray(W2, np.float32))
    b2 = np.ascontiguousarray(np.asarray(b2, np.float32))
    return [
        {"x": x[c * BL:(c + 1) * BL], "w1": W1, "b1": b1, "w2": W2, "b2": b2}
        for c in range(NCORES)
    ]


def assemble(results):
    sum1 = np.concatenate([np.asarray(r["out1"]).T for r in results], axis=0)
    sum2 = np.concatenate([np.asarray(r["out2"]).T for r in results], axis=0)
    return np.ascontiguousarray(sum1), np.ascontiguousarray(sum2)


def kernel(x, W1, b1, W2, b2, batch_size=None):
    nc = get_nc()
    in_maps = make_in_maps(x, W1, b1, W2, b2)
    res = run_bass_kernel_spmd(nc, in_maps, core_ids=list(range(NCORES)))
    return assemble(res.results)
